# revision 3
# baseline (speedup 1.0000x reference)
"""Causal multi-head attention on 8 TRN2 NeuronCores.

Problem: x[4, 2048, 2048] @ Wq/Wk/Wv[2048, 2048] -> 16-head causal attention
(head_dim 128) -> out-proj Wo[2048, 2048] + b_out.

Sharding: 4-way head tensor-parallel x 2-way batch data-parallel.
Core c handles head group (c % 4) (4 heads = 512 cols of Wq/Wk/Wv, 512 rows
of Wo) and batch pair (c // 4). Each core emits a partial out-projection for
its 2 batches; the host sums the 4 partials per batch pair (the "all-reduce")
and adds the bias.

Host prep: all inputs pre-cast to bf16 and x pre-transposed to [D, SL] per
batch pair.

Schedule (v2): the kernel is PE-bound overall, but attention alone is paced
by the Scalar engine's exp. So the phases are interleaved per batch to keep
PE saturated:
  A: projections for local batch 0 (chunks 0-3), 16 matmuls per [128,512]
     PSUM unit; chunk 0 consumes wq/xT quarter-DMAs progressively so PE
     starts ~4us in.
  B: batch-0 attention, with batch-1 projection units (48 x 16 matmuls)
     woven between score-pair groups as PE filler (4 per chunk-slot,
     slots 0-11).
  C: batch-1 attention with batch-0 out-projection tiles interleaved
     (p3i), one tile per chunk-slot, its 4 y-matmul groups also woven
     between pairs as filler.
  D: batch-1 out-projection.

Softmax denominator is computed OFF the PE: bf16 pair-sums (DVE) ->
gpsimd add tree -> one [128,512] per chunk -> DVE 32x32 stream-transpose +
grouped free-dim reduce -> [128,16] partials -> tiny DRAM bounce to fold the
4 partition blocks -> DVE reduce/reciprocal -> spread reciprocals staged to
DRAM for the out-projection normalize (PE ones-row broadcast matmul).

ctx matmuls on diagonal sk-tiles are column-restricted (sq >= sk_start);
the skipped columns are exactly the affine_select zeros.
"""

import math

import numpy as np

P = 128
S = 2048          # sequence length
D = 2048          # model dim
NB = 2            # batches per core
SL = NB * S       # local rows (4096)
DL = 512          # local head dims (4 heads x 128)
HL = 4            # local heads
NI = D // P       # 16 i-tiles
SCHUNK = 512
NCHUNK = SL // SCHUNK  # 8
SCALE = 1.0 / math.sqrt(128.0)
N_CORES = 8

_CACHE = {}


def _split_multi_waits(nc):
    """This walrus build accepts at most ONE sync-wait per instruction
    (setupSyncWait: 'Too many sync wait commands'), but Tile emits up to
    ~3 waits per instruction and the kernel-tail drain carries one wait per
    outstanding semaphore. Hoist excess waits onto single-wait nops inserted
    immediately before the instruction on the same engine stream."""
    import bass_rust

    SyncInfo = bass_rust.SyncInfo
    n = 0
    for f in nc.m.functions:
        for b in f.blocks:
            out = []
            changed = False
            for inst in list(b.instructions):
                si = getattr(inst, "sync_info", None)
                if si is not None and si.on_wait and len(si.on_wait) > 1:
                    waits = list(si.on_wait)
                    for w in waits[:-1]:
                        n += 1
                        nop = bass_rust.InstNoOp(
                            name=f"waitsplit-{n}", ins=[], outs=[]
                        )
                        nop.engine = inst.engine
                        nop.sync_info = SyncInfo(on_wait=[w], on_update=[])
                        out.append(nop)
                    inst.sync_info = SyncInfo(
                        on_wait=[waits[-1]], on_update=list(si.on_update or [])
                    )
                    changed = True
                out.append(inst)
            if changed:
                b.instructions = out


def _build():
    import concourse.bass as bass
    import concourse.mybir as mybir
    import concourse.tile as tile
    from concourse.masks import make_identity

    f32 = mybir.dt.float32
    bf16 = mybir.dt.bfloat16

    nc = bass.Bass()
    x_in = nc.declare_dram_parameter("xT", [D, SL], bf16, isOutput=False)
    wq_in = nc.declare_dram_parameter("wq", [D, DL], bf16, isOutput=False)
    wk_in = nc.declare_dram_parameter("wk", [D, DL], bf16, isOutput=False)
    wv_in = nc.declare_dram_parameter("wv", [D, DL], bf16, isOutput=False)
    wo_in = nc.declare_dram_parameter("wo", [DL, D], bf16, isOutput=False)
    y_out = nc.declare_dram_parameter("y", [SL, D], bf16, isOutput=True)

    with tile.TileContext(nc) as tc:
        _emit(nc, tc, mybir, make_identity, x_in, wq_in, wk_in, wv_in, wo_in, y_out)
    _split_multi_waits(nc)
    return nc


def _emit(nc, tc, mybir, make_identity, x_in, wq_in, wk_in, wv_in, wo_in, y_out):
    from contextlib import ExitStack

    f32 = mybir.dt.float32
    bf16 = mybir.dt.bfloat16
    Exp = mybir.ActivationFunctionType.Exp
    X = mybir.AxisListType.X
    ADD = mybir.AluOpType.add

    ctx = ExitStack()
    with ctx:
        dram = ctx.enter_context(tc.tile_pool(name="dram", bufs=1, space="DRAM"))
        consts = ctx.enter_context(tc.tile_pool(name="consts", bufs=1))
        wpool = ctx.enter_context(tc.tile_pool(name="wpool", bufs=1))
        xt_pool = ctx.enter_context(tc.tile_pool(name="xt_pool", bufs=3))
        qkv_pool = ctx.enter_context(tc.tile_pool(name="qkv_pool", bufs=3))
        att_pool = ctx.enter_context(tc.tile_pool(name="att_pool", bufs=2))
        out_pool = ctx.enter_context(tc.tile_pool(name="out_pool", bufs=3))
        # PSUM budget (8 banks of [128, 2KB]):
        #   pbig  2 x [128,1024] f32 = 4 banks  (scores; D-phase y pairs)
        #   psmall 2 x [128,512] f32 = 2 banks  (pctx; C bcp; D pya/pyb)
        #   pone  2 x [128,512] f32 = 2 banks  (P1 units; C p3i y; D bcp)
        pbig = ctx.enter_context(tc.tile_pool(name="pbig", bufs=2, space="PSUM"))
        psmall = ctx.enter_context(tc.tile_pool(name="psmall", bufs=2, space="PSUM"))
        pone = ctx.enter_context(tc.tile_pool(name="pone", bufs=2, space="PSUM"))

        # DRAM staging for q/k/v (transposed layouts), ctx, den partials
        qT_d = dram.tile([DL, SL], bf16, name="qT_d")
        kT_d = dram.tile([DL, SL], bf16, name="kT_d")
        v_d = dram.tile([P, SL // P, DL], bf16, name="v_d")
        cT_d = dram.tile([DL, SL], bf16, name="cT_d")
        recb_d = dram.tile([NB, HL, S], bf16, name="recb_d")

        qT_r = qT_d.rearrange("(a p) s -> p a s", p=P)   # [128, 4, 4096]
        kT_r = kT_d.rearrange("(a p) s -> p a s", p=P)
        v_r = v_d                                        # [128, 32, 512]
        cT_r = cT_d.rearrange("(a p) s -> p a s", p=P)

        ones1 = consts.tile([1, P], bf16, name="ones1")
        nc.vector.memset(ones1, 1.0)

        # --- weights: bf16 DMA into SBUF, 4 i-quarters each (scalar queue) ---
        wq_sb = wpool.tile([P, NI, DL], bf16, name="wq_sb")
        wk_sb = wpool.tile([P, NI, DL], bf16, name="wk_sb")
        wv_sb = wpool.tile([P, NI, DL], bf16, name="wv_sb")
        wo_sb = wpool.tile([P, HL, D], bf16, name="wo_sb")

        def emit_w(w_in, w_sb):
            w_r = w_in.rearrange("(a p) d -> p a d", p=P)  # [128, 16, 512]
            for g in range(4):
                nc.scalar.dma_start(
                    out=w_sb[:, 4 * g : 4 * g + 4, :],
                    in_=w_r[:, 4 * g : 4 * g + 4, :],
                )

        def emit_wo():
            for dt in range(HL):
                nc.scalar.dma_start(
                    out=wo_sb[:, dt, :],
                    in_=wo_in[P * dt : P * (dt + 1), :],
                )

        # --- P1 projection units (16 matmuls into one [128,512] PSUM) ---
        xT_r = x_in.rearrange("(a p) s -> p a s", p=P)  # [128, 16, 4096]
        xt_tiles = {}

        def load_xt(ch, parts=1):
            xT = xt_pool.tile([P, NI, SCHUNK], bf16, name="xT", tag="xT")
            step = NI // parts
            for g in range(parts):
                nc.sync.dma_start(
                    out=xT[:, step * g : step * (g + 1), :],
                    in_=xT_r[
                        :, step * g : step * (g + 1),
                        SCHUNK * ch : SCHUNK * (ch + 1),
                    ],
                )
            xt_tiles[ch] = xT

        def p1_qk_unit(ch, w_sb, out_r, ht):
            xT = xt_tiles[ch]
            pq = pone.tile([P, SCHUNK], f32, name="pq1", tag="po")
            for i in range(NI):
                nc.tensor.matmul(
                    pq,
                    lhsT=w_sb[:, i, P * ht : P * (ht + 1)],
                    rhs=xT[:, i, :],
                    start=(i == 0),
                    stop=(i == NI - 1),
                )
            qsb = qkv_pool.tile([P, SCHUNK], bf16, name="qsb", tag="qsb")
            nc.scalar.copy(qsb, pq)
            nc.sync.dma_start(
                out=out_r[:, ht, SCHUNK * ch : SCHUNK * (ch + 1)], in_=qsb
            )

        def p1_v_unit(ch, st):
            xT = xt_tiles[ch]
            pv = pone.tile([P, DL], f32, name="pv1", tag="po")
            for i in range(NI):
                nc.tensor.matmul(
                    pv,
                    lhsT=xT[:, i, P * st : P * (st + 1)],
                    rhs=wv_sb[:, i, :],
                    start=(i == 0),
                    stop=(i == NI - 1),
                )
            vsb = qkv_pool.tile([P, DL], bf16, name="vsb", tag="qsb")
            nc.scalar.copy(vsb, pv)
            nc.sync.dma_start(out=v_r[:, 4 * ch + st, :], in_=vsb)

        # --- Phase A: batch-0 projections (chunks 0-3) ---
        load_xt(0, parts=4)
        emit_w(wq_in, wq_sb)
        for ch in range(4):
            for ht in range(HL):
                p1_qk_unit(ch, wq_sb, qT_r, ht)
                if ch == 0 and ht == 1:
                    emit_w(wk_in, wk_sb)
            if ch + 1 < 4:
                load_xt(ch + 1)
            for ht in range(HL):
                p1_qk_unit(ch, wk_sb, kT_r, ht)
                if ch == 0 and ht == 1:
                    emit_w(wv_in, wv_sb)
                if ch == 1 and ht == 1:
                    emit_wo()
            for st in range(HL):
                p1_v_unit(ch, st)
        load_xt(4)

        # --- batch-1 projection units, drained as PE filler in phase B ---
        unit_queue = []
        for ch in range(4, NCHUNK):
            for ht in range(HL):
                unit_queue.append(("q", ch, ht))
            for ht in range(HL):
                unit_queue.append(("k", ch, ht))
            for st in range(HL):
                unit_queue.append(("v", ch, st))
        uq_pos = [0]

        def emit_unit():
            if uq_pos[0] >= len(unit_queue):
                return False
            kind, ch, j = unit_queue[uq_pos[0]]
            uq_pos[0] += 1
            if kind == "q":
                p1_qk_unit(ch, wq_sb, qT_r, j)
            elif kind == "k":
                p1_qk_unit(ch, wk_sb, kT_r, j)
            else:
                p1_v_unit(ch, j)
            return True

        # --- attention prefetch machinery ---
        bh_list = [(b, h) for b in range(NB) for h in range(HL)]
        ktb_tiles = {}
        vtb_tiles = {}

        def load_ktb(i):
            b, h = bh_list[i]
            ktb = att_pool.tile([P, S], bf16, name="ktb", tag="ktb")
            nc.sync.dma_start(
                out=ktb, in_=kT_d[P * h : P * (h + 1), S * b : S * (b + 1)]
            )
            ktb_tiles[i] = ktb

        def load_vtb(b):
            vtb_all = att_pool.tile([P, S // P, DL], bf16, name="vtb", tag="vtb")
            nc.sync.dma_start(
                out=vtb_all,
                in_=v_r[:, (S // P) * b : (S // P) * (b + 1), :],
            )
            vtb_tiles[b] = vtb_all

        qtc_tiles = {}
        cq_list = [
            (bh_i, c) for bh_i in range(len(bh_list)) for c in range(S // SCHUNK)
        ]

        def load_qtc(i):
            bh_i, c = cq_list[i]
            b, h = bh_list[bh_i]
            qtc = att_pool.tile([P, SCHUNK], bf16, name="qtc", tag="qtc", bufs=3)
            nc.sync.dma_start(
                out=qtc,
                in_=qT_d[
                    P * h : P * (h + 1),
                    S * b + SCHUNK * c : S * b + SCHUNK * (c + 1),
                ],
            )
            qtc_tiles[i] = qtc

        load_ktb(0)
        load_vtb(0)
        load_qtc(0)
        load_qtc(1)
        pend = [None]
        pend_pairs = []

        # --- P3 interleave (batch-0 out-projection during batch-1 attn) ---
        p3i_ctn = {}

        def p3i_load(t):
            ctb = out_pool.tile([P, HL, P], bf16, name="ctb", tag="ctb", bufs=4)
            nc.sync.dma_start(out=ctb, in_=cT_r[:, :, P * t : P * (t + 1)])
            rrow = out_pool.tile([1, HL * P], bf16, name="rrow", tag="rrow", bufs=4)
            nc.sync.dma_start(
                out=rrow.rearrange("q (a s) -> q a s", a=HL),
                in_=recb_d[0, :, P * t : P * (t + 1)].rearrange("a s -> () a s"),
            )
            p3i_ctn[t] = (ctb, rrow)

        def p3i_top(t):
            ctb, rrow = p3i_ctn[t]
            bcp = psmall.tile([P, 512], f32, name="p3bcp", tag="ps")
            nc.tensor.matmul(
                bcp[:, : HL * P], lhsT=ones1, rhs=rrow, start=True, stop=True
            )
            ctn = out_pool.tile([P, HL, P], bf16, name="ctn", tag="ctn", bufs=4)
            nc.vector.tensor_mul(ctn, ctb, bcp.rearrange("p (a s) -> p a s", a=HL))
            p3i_ctn[t] = ctn

        def p3i_f_thunk(t, f):
            def th():
                ctn = p3i_ctn[t]
                py = pone.tile([P, 512], f32, name="p3y", tag="po")
                for dt in range(HL):
                    nc.tensor.matmul(
                        py,
                        lhsT=ctn[:, dt, :],
                        rhs=wo_sb[:, dt, 512 * f : 512 * (f + 1)],
                        start=(dt == 0),
                        stop=(dt == HL - 1),
                    )
                ysb2 = out_pool.tile([P, 512], bf16, name="ysb2", tag="ysb2", bufs=3)
                if f % 2 == 0:
                    nc.scalar.copy(ysb2, py)
                else:
                    nc.vector.tensor_copy(ysb2, py)
                nc.sync.dma_start(
                    out=y_out[P * t : P * (t + 1), 512 * f : 512 * (f + 1)],
                    in_=ysb2,
                )
            return th

        def flush_pend():
            if pend[0] is not None:
                pend[0]()
                pend[0] = None

        # --- main attention loop (phases B and C) ---
        for bh_i, (b, h) in enumerate(bh_list):
            ktb = ktb_tiles.pop(bh_i)
            vtb_all = vtb_tiles[b]
            for c in range(S // SCHUNK):  # 4 sq-chunks
                slot = 4 * bh_i + c
                if slot in (0, 4, 8):
                    load_xt(5 + slot // 4)
                if c == 2 and bh_i + 1 < len(bh_list):
                    load_ktb(bh_i + 1)
                if bh_i == 3 and c == 0 and b + 1 < NB:
                    load_vtb(b + 1)
                cq_i = 4 * bh_i + c
                if cq_i + 2 < len(cq_list):
                    load_qtc(cq_i + 2)
                qtc = qtc_tiles.pop(cq_i)
                p3t = 4 * (bh_i - HL) + c if bh_i >= HL else None
                fillers = []
                if p3t is not None:
                    p3i_top(p3t)
                    fillers = [p3i_f_thunk(p3t, f) for f in range(4)]
                elif slot < 12:
                    fillers = [emit_unit] * 4
                pctx = psmall.tile([P, SCHUNK], f32, name="pctx", tag="ps")
                jmax = 4 * c + 4  # sk-tiles with sk_start <= sq_end
                npairs = jmax // 2
                j0s = [2 * k for k in range(npairs)]
                chunk_ds = []

                def emit_av_group(items, pctx=pctx, vtb_all=vtb_all, h=h,
                                  c=c, npairs=npairs, chunk_ds=chunk_ds):
                    # ctx matmuls back-to-back (same PSUM target), columns
                    # restricted on diagonal tiles to sq >= sk_start (the
                    # skipped columns hold affine_select zeros).
                    for at2, dsum, j0, pi in items:
                        for j2 in range(2):
                            j = j0 + j2
                            off = max(0, P * (j - 4 * c))
                            nc.tensor.matmul(
                                pctx[:, off:],
                                lhsT=vtb_all[:, j, P * h : P * (h + 1)],
                                rhs=at2[:, 512 * j2 + off : 512 * (j2 + 1)],
                                start=(pi == 0 and j2 == 0),
                                stop=(pi == npairs - 1 and j2 == 1),
                            )
                    # level-2 pair-sum on gpsimd (den tree stays off DVE/PE)
                    (_, dsa, _, _), (_, dsb, _, _) = items
                    dsum2 = att_pool.tile(
                        [P, 512], bf16, name="dsum2", tag="dsum2", bufs=4
                    )
                    nc.gpsimd.tensor_add(dsum2, dsa, dsb)
                    chunk_ds.append(dsum2)

                def emit_tail(pctx=pctx, b=b, h=h, c=c, chunk_ds=chunk_ds):
                    csb = att_pool.tile([P, SCHUNK], bf16, name="csb",
                                        tag="csb", bufs=3)
                    nc.vector.tensor_copy(csb, pctx)
                    nc.sync.dma_start(
                        out=cT_d[
                            P * h : P * (h + 1),
                            S * b + SCHUNK * c : S * b + SCHUNK * (c + 1),
                        ],
                        in_=csb,
                    )
                    # gpsimd tree -> one [128,512] of sk-partials
                    lvl = list(chunk_ds)
                    while len(lvl) > 1:
                        nxt = []
                        for k in range(0, len(lvl) - 1, 2):
                            t = att_pool.tile([P, 512], bf16, name="dtree",
                                              tag="dtree", bufs=3)
                            nc.gpsimd.tensor_add(t, lvl[k], lvl[k + 1])
                            nxt.append(t)
                        if len(lvl) % 2:
                            nxt.append(lvl[-1])
                        lvl = nxt
                    dtot = lvl[0]
                    # DVE: 32x32 block transpose + grouped free reduce
                    # -> z[32a+u, g] = sum over partition block a of
                    #    column q = 32g+u
                    yst = att_pool.tile([P, 512], bf16, name="yst",
                                        tag="yst", bufs=2)
                    nc.vector.transpose(yst, dtot)
                    z = att_pool.tile([P, 16], f32, name="zred",
                                      tag="zred", bufs=2)
                    nc.vector.tensor_reduce(
                        z, yst.rearrange("p (g u) -> p g u", u=32),
                        axis=X, op=ADD,
                    )
                    # fold the 4 partition blocks via a tiny DRAM bounce
                    # (a-major layout: 64B contiguous runs both ways)
                    zd = dram.tile([2048], f32, name="zd", tag="zd", bufs=2)
                    nc.sync.dma_start(
                        out=zd.rearrange("(a u g) -> (a u) g", a=4, u=32),
                        in_=z,
                    )
                    zt = att_pool.tile([32, 4, 16], f32, name="zt",
                                       tag="zt", bufs=2)
                    nc.sync.dma_start(
                        out=zt, in_=zd.rearrange("(a u g) -> u a g", a=4, u=32)
                    )
                    den_sp = att_pool.tile([32, 16], f32, name="den_sp",
                                           tag="den_sp", bufs=2)
                    nc.vector.tensor_reduce(
                        den_sp, zt.rearrange("u a g -> u g a"), axis=X, op=ADD
                    )
                    rsp = att_pool.tile([32, 16], f32, name="rsp",
                                        tag="rsp", bufs=2)
                    nc.vector.reciprocal(rsp, den_sp)
                    rspb = att_pool.tile([32, 16], bf16, name="rspb",
                                         tag="rspb", bufs=2)
                    nc.vector.tensor_copy(rspb, rsp)
                    nc.sync.dma_start(
                        out=recb_d[
                            b, h, SCHUNK * c : SCHUNK * (c + 1)
                        ].rearrange("(g p) -> p g", p=32),
                        in_=rspb,
                    )

                for pi, j0 in enumerate(j0s):
                    ps2 = pbig.tile([P, 1024], f32, name="ps2", tag="pb")
                    for j2 in range(2):
                        j = j0 + j2
                        off = max(0, P * (j - 4 * c))
                        nc.tensor.matmul(
                            ps2[:, 512 * j2 + off : 512 * (j2 + 1)],
                            lhsT=ktb[:, P * j : P * (j + 1)],
                            rhs=qtc[:, off:],
                            start=True,
                            stop=True,
                        )
                    at2 = att_pool.tile([P, 1024], bf16, name="at2",
                                        tag="at2", bufs=5)
                    nc.scalar.activation(at2, ps2, Exp, scale=SCALE)
                    if j0 >= 4 * c:  # diagonal pair: zero sk > sq
                        nc.gpsimd.affine_select(
                            out=at2.rearrange("p (a b) -> p a b", a=2),
                            in_=at2.rearrange("p (a b) -> p a b", a=2),
                            compare_op=mybir.AluOpType.is_ge,
                            fill=0.0,
                            base=(0 if j0 == 4 * c else -256),
                            channel_multiplier=-1,
                            pattern=[[-P, 2], [1, 512]],
                        )
                    dsum = att_pool.tile([P, 512], bf16, name="dsum",
                                         tag="dsum", bufs=5)
                    nc.vector.tensor_add(dsum, at2[:, :512], at2[:, 512:])
                    # PE filler between pair groups: batch-1 projection
                    # units (B) or p3i y-matmul groups (C) cover the exp
                    # latency of the deferred ctx group.
                    if fillers:
                        fillers.pop(0)()
                    flush_pend()
                    pend_pairs.append((at2, dsum, j0, pi))
                    if len(pend_pairs) == 2:
                        items = list(pend_pairs)
                        pend_pairs.clear()
                        is_last = pi == npairs - 1

                        def pend_fn(items=items, emit=emit_av_group,
                                    tail=(emit_tail if is_last else None)):
                            emit(items)
                            if tail is not None:
                                tail()

                        pend[0] = pend_fn
                for th in fillers:
                    th()
                nxt = 0 if (bh_i == HL - 1 and c == 3) else (
                    p3t + 1 if p3t is not None else None
                )
                if nxt is not None and nxt < S // P:
                    p3i_load(nxt)
        flush_pend()

        # --- Phase D: batch-1 out-projection ---
        bt_list = [(1, t) for t in range(S // P)]

        def p3_load(i):
            b, t = bt_list[i]
            col0 = S * b + P * t
            ctb = out_pool.tile([P, HL, P], bf16, name="ctb", tag="ctb", bufs=4)
            nc.sync.dma_start(out=ctb, in_=cT_r[:, :, col0 : col0 + P])
            rrow = out_pool.tile([1, HL * P], bf16, name="rrow", tag="rrow", bufs=4)
            nc.sync.dma_start(
                out=rrow.rearrange("q (a s) -> q a s", a=HL),
                in_=recb_d[b, :, P * t : P * (t + 1)].rearrange("a s -> () a s"),
            )
            bcp = pone.tile([P, 512], f32, name="bcp", tag="po")
            nc.tensor.matmul(
                bcp[:, : HL * P], lhsT=ones1, rhs=rrow, start=True, stop=True
            )
            ctn = out_pool.tile([P, HL, P], bf16, name="ctn", tag="ctn", bufs=4)
            nc.vector.tensor_mul(
                ctn, ctb, bcp.rearrange("p (a s) -> p a s", a=HL)
            )
            return ctn

        ctn_next = p3_load(0)
        for i, (b, t) in enumerate(bt_list):
            ctn = ctn_next
            if i + 1 < len(bt_list):
                ctn_next = p3_load(i + 1)
            col0 = S * b + P * t
            use_small = (i % 2 == 0)
            for fp in range(2):  # pairs of f-chunks
                if use_small:
                    pya = psmall.tile([P, 512], f32, name="pya", tag="ps")
                    pyb = psmall.tile([P, 512], f32, name="pyb", tag="ps")
                    halves = (pya, pyb)
                else:
                    py = pbig.tile([P, 1024], f32, name="py", tag="pb")
                    halves = (py[:, :512], py[:, 512:])
                for f2 in range(2):
                    f = 2 * fp + f2
                    for dt in range(HL):
                        nc.tensor.matmul(
                            halves[f2],
                            lhsT=ctn[:, dt, :],
                            rhs=wo_sb[:, dt, 512 * f : 512 * (f + 1)],
                            start=(dt == 0),
                            stop=(dt == HL - 1),
                        )
                ysb = out_pool.tile([P, 1024], bf16, name="ysb", tag="ysb", bufs=3)
                if fp == 0:
                    nc.vector.tensor_copy(ysb[:, :512], halves[0])
                    nc.vector.tensor_copy(ysb[:, 512:], halves[1])
                else:
                    nc.scalar.copy(ysb[:, :512], halves[0])
                    nc.scalar.copy(ysb[:, 512:], halves[1])
                nc.scalar.dma_start(
                    out=y_out[col0 : col0 + P, 1024 * fp : 1024 * (fp + 1)],
                    in_=ysb,
                )


def _get_nc():
    if "nc" not in _CACHE:
        _CACHE["nc"] = _build()
    return _CACHE["nc"]


def _run(inputs, trace=False):
    import ml_dtypes

    from concourse.bass_utils import run_bass_kernel_spmd

    bf = ml_dtypes.bfloat16
    x = np.asarray(inputs["x"], dtype=np.float32).astype(bf)
    wq = np.asarray(inputs["W_query"], dtype=np.float32).astype(bf)
    wk = np.asarray(inputs["W_key"], dtype=np.float32).astype(bf)
    wv = np.asarray(inputs["W_value"], dtype=np.float32).astype(bf)
    wo = np.asarray(inputs["W_out"], dtype=np.float32).astype(bf)
    b_out = np.asarray(inputs["b_out"], dtype=np.float32)

    xf = x.reshape(2, SL, D)  # batch pairs
    in_maps = []
    for c in range(N_CORES):
        pair = c // 4
        hg = c % 4
        in_maps.append(
            {
                "xT": np.ascontiguousarray(xf[pair].T),
                "wq": np.ascontiguousarray(wq[:, DL * hg : DL * (hg + 1)]),
                "wk": np.ascontiguousarray(wk[:, DL * hg : DL * (hg + 1)]),
                "wv": np.ascontiguousarray(wv[:, DL * hg : DL * (hg + 1)]),
                "wo": np.ascontiguousarray(wo[DL * hg : DL * (hg + 1), :]),
            }
        )

    nc = _get_nc()
    res = run_bass_kernel_spmd(nc, in_maps, core_ids=list(range(N_CORES)), trace=trace)

    y = np.zeros((2, SL, D), dtype=np.float32)
    for c in range(N_CORES):
        y[c // 4] += res.results[c]["y"].astype(np.float32)
    y += b_out[None, None, :]
    out = y.reshape(4, S, D)
    return out, res


def kernel(**inputs) -> np.ndarray:
    out, _ = _run(inputs, trace=False)
    return out


# revision 8
# speedup vs baseline: 1.2330x; 1.2330x over previous
"""Causal multi-head attention on 8 TRN2 NeuronCores.

Problem: x[4, 2048, 2048] @ Wq/Wk/Wv[2048, 2048] -> 16-head causal attention
(head_dim 128) -> out-proj Wo[2048, 2048] + b_out.

Sharding: 4-way head tensor-parallel x 2-way batch data-parallel.
Core c handles head group (c % 4) (4 heads = 512 cols of Wq/Wk/Wv, 512 rows
of Wo) and batch pair (c // 4). Each core emits a partial out-projection for
its 2 batches; the host sums the 4 partials per batch pair (the "all-reduce")
and adds the bias.

Host prep: all inputs pre-cast to bf16 and x pre-transposed to [D, SL] per
batch pair.

Schedule (v2): the kernel is PE-bound overall, but attention alone is paced
by the Scalar engine's exp. So the phases are interleaved per batch to keep
PE saturated:
  A: projections for local batch 0 (chunks 0-3), 16 matmuls per [128,512]
     PSUM unit; chunk 0 consumes wq/xT quarter-DMAs progressively so PE
     starts ~4us in.
  B: batch-0 attention, with batch-1 projection units (48 x 16 matmuls)
     woven between score-pair groups as PE filler (4 per chunk-slot,
     slots 0-11).
  C: batch-1 attention with batch-0 out-projection tiles interleaved
     (p3i), one tile per chunk-slot, its 4 y-matmul groups also woven
     between pairs as filler.
  D: batch-1 out-projection.

Softmax denominator is computed OFF the PE: bf16 pair-sums (DVE) ->
gpsimd add tree -> one [128,512] per chunk -> DVE 32x32 stream-transpose +
grouped free-dim reduce -> [128,16] partials -> tiny DRAM bounce to fold the
4 partition blocks -> DVE reduce/reciprocal -> spread reciprocals staged to
DRAM for the out-projection normalize (PE ones-row broadcast matmul).

ctx matmuls on diagonal sk-tiles are column-restricted (sq >= sk_start);
the skipped columns are exactly the affine_select zeros.
"""

import math

import numpy as np

P = 128
S = 2048          # sequence length
D = 2048          # model dim
NB = 2            # batches per core
SL = NB * S       # local rows (4096)
DL = 512          # local head dims (4 heads x 128)
HL = 4            # local heads
NI = D // P       # 16 i-tiles
SCHUNK = 512
NCHUNK = SL // SCHUNK  # 8
SCALE = 1.0 / math.sqrt(128.0)
N_CORES = 8

_CACHE = {}


def _split_multi_waits(nc):
    """This walrus build accepts at most ONE sync-wait per instruction
    (setupSyncWait: 'Too many sync wait commands'), but Tile emits up to
    ~3 waits per instruction and the kernel-tail drain carries one wait per
    outstanding semaphore. Hoist excess waits onto single-wait nops inserted
    immediately before the instruction on the same engine stream."""
    import bass_rust

    SyncInfo = bass_rust.SyncInfo
    n = 0
    for f in nc.m.functions:
        for b in f.blocks:
            out = []
            changed = False
            for inst in list(b.instructions):
                si = getattr(inst, "sync_info", None)
                if si is not None and si.on_wait and len(si.on_wait) > 1:
                    waits = list(si.on_wait)
                    for w in waits[:-1]:
                        n += 1
                        nop = bass_rust.InstNoOp(
                            name=f"waitsplit-{n}", ins=[], outs=[]
                        )
                        nop.engine = inst.engine
                        nop.sync_info = SyncInfo(on_wait=[w], on_update=[])
                        out.append(nop)
                    inst.sync_info = SyncInfo(
                        on_wait=[waits[-1]], on_update=list(si.on_update or [])
                    )
                    changed = True
                out.append(inst)
            if changed:
                b.instructions = out


def _build():
    import concourse.bass as bass
    import concourse.mybir as mybir
    import concourse.tile as tile
    from concourse.masks import make_identity

    f32 = mybir.dt.float32
    bf16 = mybir.dt.bfloat16

    nc = bass.Bass()
    x_in = nc.declare_dram_parameter("xT", [D, SL], bf16, isOutput=False)
    wq_in = nc.declare_dram_parameter("wq", [D, DL], bf16, isOutput=False)
    wk_in = nc.declare_dram_parameter("wk", [D, DL], bf16, isOutput=False)
    wv_in = nc.declare_dram_parameter("wv", [D, DL], bf16, isOutput=False)
    wo_in = nc.declare_dram_parameter("wo", [DL, D], bf16, isOutput=False)
    y_out = nc.declare_dram_parameter("y", [SL, D], bf16, isOutput=True)

    with tile.TileContext(nc) as tc:
        _emit(nc, tc, mybir, make_identity, x_in, wq_in, wk_in, wv_in, wo_in, y_out)
    _split_multi_waits(nc)
    return nc


def _emit(nc, tc, mybir, make_identity, x_in, wq_in, wk_in, wv_in, wo_in, y_out):
    from contextlib import ExitStack

    f32 = mybir.dt.float32
    bf16 = mybir.dt.bfloat16
    Exp = mybir.ActivationFunctionType.Exp
    X = mybir.AxisListType.X
    ADD = mybir.AluOpType.add

    ctx = ExitStack()
    with ctx:
        dram = ctx.enter_context(tc.tile_pool(name="dram", bufs=1, space="DRAM"))
        consts = ctx.enter_context(tc.tile_pool(name="consts", bufs=1))
        wpool = ctx.enter_context(tc.tile_pool(name="wpool", bufs=1))
        xt_pool = ctx.enter_context(tc.tile_pool(name="xt_pool", bufs=3))
        qkv_pool = ctx.enter_context(tc.tile_pool(name="qkv_pool", bufs=3))
        att_pool = ctx.enter_context(tc.tile_pool(name="att_pool", bufs=2))
        out_pool = ctx.enter_context(tc.tile_pool(name="out_pool", bufs=3))
        # PSUM budget (8 banks of [128, 2KB]):
        #   pbig  2 x [128,1024] f32 = 4 banks  (scores; D-phase y pairs)
        #   psmall 2 x [128,512] f32 = 2 banks  (pctx; C bcp; D pya/pyb)
        #   pone  2 x [128,512] f32 = 2 banks  (P1 units; C p3i y; D bcp)
        pbig = ctx.enter_context(tc.tile_pool(name="pbig", bufs=2, space="PSUM"))
        psmall = ctx.enter_context(tc.tile_pool(name="psmall", bufs=2, space="PSUM"))
        pone = ctx.enter_context(tc.tile_pool(name="pone", bufs=2, space="PSUM"))

        # DRAM staging for q/k/v (transposed layouts), ctx, den partials
        qT_d = dram.tile([DL, SL], bf16, name="qT_d")
        kT_d = dram.tile([DL, SL], bf16, name="kT_d")
        v_d = dram.tile([P, SL // P, DL], bf16, name="v_d")
        cT_d = dram.tile([DL, SL], bf16, name="cT_d")
        recb_d = dram.tile([NB, HL, S], bf16, name="recb_d")

        qT_r = qT_d.rearrange("(a p) s -> p a s", p=P)   # [128, 4, 4096]
        kT_r = kT_d.rearrange("(a p) s -> p a s", p=P)
        v_r = v_d                                        # [128, 32, 512]
        cT_r = cT_d.rearrange("(a p) s -> p a s", p=P)

        ones1 = consts.tile([1, P], bf16, name="ones1")
        nc.vector.memset(ones1, 1.0)
        # fold mask M[p, u] = 1 iff p %% 32 == u (for the den partition-block
        # fold matmul): sum the 32-col groups of a 128x128 identity.
        ident, ident_free = tc.tile([P, P], bf16, name="ident")
        make_identity(nc, ident)
        mfold_f, mfold_f_free = tc.tile([P, 32], f32, name="mfold_f")
        nc.vector.tensor_reduce(
            mfold_f, ident.rearrange("p (a u) -> p u a", a=4), axis=X, op=ADD
        )
        mfold = consts.tile([P, 32], bf16, name="mfold")
        nc.vector.tensor_copy(mfold, mfold_f)
        mfold_f_free()
        ident_free()

        # --- weights: bf16 DMA into SBUF, 4 i-quarters each (scalar queue) ---
        wq_sb = wpool.tile([P, NI, DL], bf16, name="wq_sb")
        wk_sb = wpool.tile([P, NI, DL], bf16, name="wk_sb")
        wv_sb = wpool.tile([P, NI, DL], bf16, name="wv_sb")
        wo_sb = wpool.tile([P, HL, D], bf16, name="wo_sb")

        def emit_w(w_in, w_sb):
            w_r = w_in.rearrange("(a p) d -> p a d", p=P)  # [128, 16, 512]
            for g in range(4):
                nc.scalar.dma_start(
                    out=w_sb[:, 4 * g : 4 * g + 4, :],
                    in_=w_r[:, 4 * g : 4 * g + 4, :],
                )

        def emit_wo():
            for dt in range(HL):
                nc.scalar.dma_start(
                    out=wo_sb[:, dt, :],
                    in_=wo_in[P * dt : P * (dt + 1), :],
                )

        # --- P1 projection units (16 matmuls into one [128,512] PSUM) ---
        xT_r = x_in.rearrange("(a p) s -> p a s", p=P)  # [128, 16, 4096]
        xt_tiles = {}

        def load_xt(ch, parts=1):
            xT = xt_pool.tile([P, NI, SCHUNK], bf16, name="xT", tag="xT")
            step = NI // parts
            for g in range(parts):
                nc.sync.dma_start(
                    out=xT[:, step * g : step * (g + 1), :],
                    in_=xT_r[
                        :, step * g : step * (g + 1),
                        SCHUNK * ch : SCHUNK * (ch + 1),
                    ],
                )
            xt_tiles[ch] = xT

        def p1_qk_unit(ch, w_sb, out_r, ht):
            xT = xt_tiles[ch]
            pq = pone.tile([P, SCHUNK], f32, name="pq1", tag="po")
            for i in range(NI):
                nc.tensor.matmul(
                    pq,
                    lhsT=w_sb[:, i, P * ht : P * (ht + 1)],
                    rhs=xT[:, i, :],
                    start=(i == 0),
                    stop=(i == NI - 1),
                )
            qsb = qkv_pool.tile([P, SCHUNK], bf16, name="qsb", tag="qsb", bufs=3)
            nc.scalar.copy(qsb, pq)
            nc.scalar.dma_start(
                out=out_r[:, ht, SCHUNK * ch : SCHUNK * (ch + 1)], in_=qsb
            )

        def p1_v_unit(ch, st):
            xT = xt_tiles[ch]
            pv = pone.tile([P, DL], f32, name="pv1", tag="po")
            for i in range(NI):
                nc.tensor.matmul(
                    pv,
                    lhsT=xT[:, i, P * st : P * (st + 1)],
                    rhs=wv_sb[:, i, :],
                    start=(i == 0),
                    stop=(i == NI - 1),
                )
            vsb = qkv_pool.tile([P, DL], bf16, name="vsb", tag="qsb", bufs=3)
            nc.scalar.copy(vsb, pv)
            nc.scalar.dma_start(out=v_r[:, 4 * ch + st, :], in_=vsb)

        # --- Phase A: batch-0 projections (chunks 0-3) ---
        load_xt(0, parts=4)
        emit_w(wq_in, wq_sb)
        for ch in range(4):
            for ht in range(HL):
                p1_qk_unit(ch, wq_sb, qT_r, ht)
                if ch == 0 and ht == 1:
                    emit_w(wk_in, wk_sb)
            if ch + 1 < 4:
                load_xt(ch + 1)
            for ht in range(HL):
                p1_qk_unit(ch, wk_sb, kT_r, ht)
                if ch == 0 and ht == 1:
                    emit_w(wv_in, wv_sb)
                if ch == 1 and ht == 1:
                    emit_wo()
            for st in range(HL):
                p1_v_unit(ch, st)
        load_xt(4)

        # --- batch-1 projection units, drained as PE filler in phase B ---
        unit_queue = []
        for ch in range(4, NCHUNK):
            for ht in range(HL):
                unit_queue.append(("q", ch, ht))
            for ht in range(HL):
                unit_queue.append(("k", ch, ht))
            for st in range(HL):
                unit_queue.append(("v", ch, st))
        uq_pos = [0]

        def emit_unit():
            if uq_pos[0] >= len(unit_queue):
                return False
            kind, ch, j = unit_queue[uq_pos[0]]
            uq_pos[0] += 1
            if kind == "q":
                p1_qk_unit(ch, wq_sb, qT_r, j)
            elif kind == "k":
                p1_qk_unit(ch, wk_sb, kT_r, j)
            else:
                p1_v_unit(ch, j)
            return True

        # --- attention prefetch machinery ---
        bh_list = [(b, h) for b in range(NB) for h in range(HL)]
        ktb_tiles = {}
        vtb_tiles = {}

        def load_ktb(i):
            b, h = bh_list[i]
            ktb = att_pool.tile([P, S], bf16, name="ktb", tag="ktb")
            nc.sync.dma_start(
                out=ktb, in_=kT_d[P * h : P * (h + 1), S * b : S * (b + 1)]
            )
            ktb_tiles[i] = ktb

        def load_vtb(b):
            vtb_all = att_pool.tile([P, S // P, DL], bf16, name="vtb", tag="vtb")
            nc.sync.dma_start(
                out=vtb_all,
                in_=v_r[:, (S // P) * b : (S // P) * (b + 1), :],
            )
            vtb_tiles[b] = vtb_all

        qtc_tiles = {}
        cq_list = [
            (bh_i, c) for bh_i in range(len(bh_list)) for c in range(S // SCHUNK)
        ]

        def load_qtc(i):
            bh_i, c = cq_list[i]
            b, h = bh_list[bh_i]
            qtc = att_pool.tile([P, SCHUNK], bf16, name="qtc", tag="qtc", bufs=3)
            nc.sync.dma_start(
                out=qtc,
                in_=qT_d[
                    P * h : P * (h + 1),
                    S * b + SCHUNK * c : S * b + SCHUNK * (c + 1),
                ],
            )
            qtc_tiles[i] = qtc

        load_ktb(0)
        load_vtb(0)
        load_qtc(0)
        load_qtc(1)
        pend = [None]
        pend_pairs = []
        pend_late = []

        # --- P3 interleave (batch-0 out-projection during batch-1 attn) ---
        p3i_ctn = {}

        def p3i_load(t):
            ctb = out_pool.tile([P, HL, P], bf16, name="ctb", tag="ctb", bufs=4)
            nc.sync.dma_start(out=ctb, in_=cT_r[:, :, P * t : P * (t + 1)])
            rrow = out_pool.tile([1, HL * P], bf16, name="rrow", tag="rrow", bufs=4)
            nc.sync.dma_start(
                out=rrow.rearrange("q (a s) -> q a s", a=HL),
                in_=recb_d[0, :, P * t : P * (t + 1)].rearrange("a s -> () a s"),
            )
            p3i_ctn[t] = (ctb, rrow)

        def p3i_top(t):
            ctb, rrow = p3i_ctn[t]
            bcp = psmall.tile([P, 512], f32, name="p3bcp", tag="ps")
            nc.tensor.matmul(
                bcp[:, : HL * P], lhsT=ones1, rhs=rrow, start=True, stop=True
            )
            ctn = out_pool.tile([P, HL, P], bf16, name="ctn", tag="ctn", bufs=4)
            nc.vector.tensor_mul(ctn, ctb, bcp.rearrange("p (a s) -> p a s", a=HL))
            p3i_ctn[t] = ctn

        def p3i_f_thunk(t, f):
            def th():
                ctn = p3i_ctn[t]
                py = pone.tile([P, 512], f32, name="p3y", tag="po")
                for dt in range(HL):
                    nc.tensor.matmul(
                        py,
                        lhsT=ctn[:, dt, :],
                        rhs=wo_sb[:, dt, 512 * f : 512 * (f + 1)],
                        start=(dt == 0),
                        stop=(dt == HL - 1),
                    )
                ysb2 = out_pool.tile([P, 512], bf16, name="ysb2", tag="ysb2", bufs=3)
                if f % 2 == 0:
                    nc.scalar.copy(ysb2, py)
                    nc.scalar.dma_start(
                        out=y_out[P * t : P * (t + 1), 512 * f : 512 * (f + 1)],
                        in_=ysb2,
                    )
                else:
                    nc.vector.tensor_copy(ysb2, py)
                    nc.sync.dma_start(
                        out=y_out[P * t : P * (t + 1), 512 * f : 512 * (f + 1)],
                        in_=ysb2,
                    )
            return th

        def flush_pend():
            # late tails (reciprocal path) run one flush after their tail so
            # the PE fold matmul and the engines feeding it are never waited on
            late = list(pend_late)
            pend_late.clear()
            if pend[0] is not None:
                pend[0]()
                pend[0] = None
            for fn in late:
                fn()

        # --- main attention loop (phases B and C) ---
        for bh_i, (b, h) in enumerate(bh_list):
            ktb = ktb_tiles.pop(bh_i)
            vtb_all = vtb_tiles[b]
            for c in range(S // SCHUNK):  # 4 sq-chunks
                slot = 4 * bh_i + c
                if slot in (0, 4, 8):
                    load_xt(5 + slot // 4)
                if c == 2 and bh_i + 1 < len(bh_list):
                    load_ktb(bh_i + 1)
                if bh_i == 3 and c == 0 and b + 1 < NB:
                    load_vtb(b + 1)
                cq_i = 4 * bh_i + c
                if cq_i + 2 < len(cq_list):
                    load_qtc(cq_i + 2)
                qtc = qtc_tiles.pop(cq_i)
                p3t = 4 * (bh_i - HL) + c if bh_i >= HL else None
                fillers = []
                if p3t is not None:
                    p3i_top(p3t)
                    fillers = [p3i_f_thunk(p3t, f) for f in range(4)]
                elif slot < 12:
                    fillers = [emit_unit] * 4
                pctx = psmall.tile([P, SCHUNK], f32, name="pctx", tag="ps")
                jmax = 4 * c + 4  # sk-tiles with sk_start <= sq_end
                npairs = jmax // 2
                j0s = [2 * k for k in range(npairs)]
                chunk_ds = []

                def emit_av_group(items, pctx=pctx, vtb_all=vtb_all, h=h,
                                  c=c, npairs=npairs, chunk_ds=chunk_ds):
                    # ctx matmuls back-to-back (same PSUM target), columns
                    # restricted on diagonal tiles to sq >= sk_start (the
                    # skipped columns hold affine_select zeros).
                    for at2, dsum, j0, pi in items:
                        for j2 in range(2):
                            j = j0 + j2
                            off = max(0, P * (j - 4 * c))
                            nc.tensor.matmul(
                                pctx[:, off:],
                                lhsT=vtb_all[:, j, P * h : P * (h + 1)],
                                rhs=at2[:, 512 * j2 + off : 512 * (j2 + 1)],
                                start=(pi == 0 and j2 == 0),
                                stop=(pi == npairs - 1 and j2 == 1),
                            )
                    # level-2 pair-sum on gpsimd (den tree stays off DVE/PE)
                    (_, dsa, _, _), (_, dsb, _, _) = items
                    dsum2 = att_pool.tile(
                        [P, 512], bf16, name="dsum2", tag="dsum2", bufs=4
                    )
                    nc.gpsimd.tensor_add(dsum2, dsa, dsb)
                    chunk_ds.append(dsum2)

                def emit_tail(pctx=pctx, b=b, h=h, c=c, chunk_ds=chunk_ds):
                    csb = att_pool.tile([P, SCHUNK], bf16, name="csb",
                                        tag="csb", bufs=3)
                    nc.scalar.copy(csb, pctx)
                    nc.scalar.dma_start(
                        out=cT_d[
                            P * h : P * (h + 1),
                            S * b + SCHUNK * c : S * b + SCHUNK * (c + 1),
                        ],
                        in_=csb,
                    )
                    # gpsimd tree -> one [128,512] of sk-partials
                    lvl = list(chunk_ds)
                    while len(lvl) > 1:
                        nxt = []
                        for k in range(0, len(lvl) - 1, 2):
                            t = att_pool.tile([P, 512], bf16, name="dtree",
                                              tag="dtree", bufs=3)
                            nc.gpsimd.tensor_add(t, lvl[k], lvl[k + 1])
                            nxt.append(t)
                        if len(lvl) % 2:
                            nxt.append(lvl[-1])
                        lvl = nxt
                    dtot = lvl[0]
                    # DVE: 32x32 block transpose + grouped free reduce
                    # -> z[32a+u, g] = sum over partition block a of
                    #    column q = 32g+u
                    yst = att_pool.tile([P, 512], bf16, name="yst",
                                        tag="yst", bufs=2)
                    nc.vector.transpose(yst, dtot)
                    z = att_pool.tile([P, 16], f32, name="zred",
                                      tag="zred", bufs=2)
                    nc.vector.tensor_reduce(
                        z, yst.rearrange("p (g u) -> p g u", u=32),
                        axis=X, op=ADD,
                    )
                    zb = att_pool.tile([P, 16], bf16, name="zb",
                                       tag="zb", bufs=2)
                    nc.scalar.copy(zb, z)
                    # fold the 4 partition blocks on the PE: a 16-column
                    # mask matmul (den_ps[u, g] = sum_a z[32a+u, g])
                    den_ps = pone.tile([32, 16], f32, name="den_ps", tag="po")
                    nc.tensor.matmul(
                        den_ps, lhsT=mfold, rhs=zb, start=True, stop=True
                    )

                    def tail_late(den_ps=den_ps, b=b, h=h, c=c):
                        rsp = att_pool.tile([32, 16], f32, name="rsp",
                                            tag="rsp", bufs=2)
                        nc.vector.reciprocal(rsp, den_ps)
                        rspb = att_pool.tile([32, 16], bf16, name="rspb",
                                             tag="rspb", bufs=2)
                        nc.vector.tensor_copy(rspb, rsp)
                        nc.sync.dma_start(
                            out=recb_d[
                                b, h, SCHUNK * c : SCHUNK * (c + 1)
                            ].rearrange("(g p) -> p g", p=32),
                            in_=rspb,
                        )
                    pend_late.append(tail_late)

                for pi, j0 in enumerate(j0s):
                    ps2 = pbig.tile([P, 1024], f32, name="ps2", tag="pb")
                    for j2 in range(2):
                        j = j0 + j2
                        off = max(0, P * (j - 4 * c))
                        nc.tensor.matmul(
                            ps2[:, 512 * j2 + off : 512 * (j2 + 1)],
                            lhsT=ktb[:, P * j : P * (j + 1)],
                            rhs=qtc[:, off:],
                            start=True,
                            stop=True,
                        )
                    at2 = att_pool.tile([P, 1024], bf16, name="at2",
                                        tag="at2", bufs=5)
                    nc.scalar.activation(at2, ps2, Exp, scale=SCALE)
                    if j0 >= 4 * c:  # diagonal pair: zero sk > sq
                        nc.gpsimd.affine_select(
                            out=at2.rearrange("p (a b) -> p a b", a=2),
                            in_=at2.rearrange("p (a b) -> p a b", a=2),
                            compare_op=mybir.AluOpType.is_ge,
                            fill=0.0,
                            base=(0 if j0 == 4 * c else -256),
                            channel_multiplier=-1,
                            pattern=[[-P, 2], [1, 512]],
                        )
                    dsum = att_pool.tile([P, 512], bf16, name="dsum",
                                         tag="dsum", bufs=5)
                    nc.vector.tensor_add(dsum, at2[:, :512], at2[:, 512:])
                    # PE filler between pair groups: batch-1 projection
                    # units (B) or p3i y-matmul groups (C) cover the exp
                    # latency of the deferred ctx group.
                    if fillers:
                        fillers.pop(0)()
                    flush_pend()
                    pend_pairs.append((at2, dsum, j0, pi))
                    if len(pend_pairs) == 2:
                        items = list(pend_pairs)
                        pend_pairs.clear()
                        is_last = pi == npairs - 1

                        def pend_fn(items=items, emit=emit_av_group,
                                    tail=(emit_tail if is_last else None)):
                            emit(items)
                            if tail is not None:
                                tail()

                        pend[0] = pend_fn
                for th in fillers:
                    th()
                nxt = 0 if (bh_i == HL - 1 and c == 3) else (
                    p3t + 1 if p3t is not None else None
                )
                if nxt is not None and nxt < S // P:
                    p3i_load(nxt)
        flush_pend()
        flush_pend()

        # --- Phase D: batch-1 out-projection ---
        bt_list = [(1, t) for t in range(S // P)]

        def p3_load(i):
            b, t = bt_list[i]
            col0 = S * b + P * t
            ctb = out_pool.tile([P, HL, P], bf16, name="ctb", tag="ctb", bufs=4)
            nc.sync.dma_start(out=ctb, in_=cT_r[:, :, col0 : col0 + P])
            rrow = out_pool.tile([1, HL * P], bf16, name="rrow", tag="rrow", bufs=4)
            nc.sync.dma_start(
                out=rrow.rearrange("q (a s) -> q a s", a=HL),
                in_=recb_d[b, :, P * t : P * (t + 1)].rearrange("a s -> () a s"),
            )
            bcp = pone.tile([P, 512], f32, name="bcp", tag="po")
            nc.tensor.matmul(
                bcp[:, : HL * P], lhsT=ones1, rhs=rrow, start=True, stop=True
            )
            ctn = out_pool.tile([P, HL, P], bf16, name="ctn", tag="ctn", bufs=4)
            nc.vector.tensor_mul(
                ctn, ctb, bcp.rearrange("p (a s) -> p a s", a=HL)
            )
            return ctn

        ctn_next = p3_load(0)
        for i, (b, t) in enumerate(bt_list):
            ctn = ctn_next
            if i + 1 < len(bt_list):
                ctn_next = p3_load(i + 1)
            col0 = S * b + P * t
            use_small = (i % 2 == 0)
            for fp in range(2):  # pairs of f-chunks
                if use_small:
                    pya = psmall.tile([P, 512], f32, name="pya", tag="ps")
                    pyb = psmall.tile([P, 512], f32, name="pyb", tag="ps")
                    halves = (pya, pyb)
                else:
                    py = pbig.tile([P, 1024], f32, name="py", tag="pb")
                    halves = (py[:, :512], py[:, 512:])
                for f2 in range(2):
                    f = 2 * fp + f2
                    for dt in range(HL):
                        nc.tensor.matmul(
                            halves[f2],
                            lhsT=ctn[:, dt, :],
                            rhs=wo_sb[:, dt, 512 * f : 512 * (f + 1)],
                            start=(dt == 0),
                            stop=(dt == HL - 1),
                        )
                ysb = out_pool.tile([P, 1024], bf16, name="ysb", tag="ysb", bufs=3)
                if fp == 0:
                    nc.vector.tensor_copy(ysb[:, :512], halves[0])
                    nc.vector.tensor_copy(ysb[:, 512:], halves[1])
                else:
                    nc.scalar.copy(ysb[:, :512], halves[0])
                    nc.scalar.copy(ysb[:, 512:], halves[1])
                nc.scalar.dma_start(
                    out=y_out[col0 : col0 + P, 1024 * fp : 1024 * (fp + 1)],
                    in_=ysb,
                )


def _get_nc():
    if "nc" not in _CACHE:
        _CACHE["nc"] = _build()
    return _CACHE["nc"]


def _run(inputs, trace=False):
    import ml_dtypes

    from concourse.bass_utils import run_bass_kernel_spmd

    bf = ml_dtypes.bfloat16
    x = np.asarray(inputs["x"], dtype=np.float32).astype(bf)
    wq = np.asarray(inputs["W_query"], dtype=np.float32).astype(bf)
    wk = np.asarray(inputs["W_key"], dtype=np.float32).astype(bf)
    wv = np.asarray(inputs["W_value"], dtype=np.float32).astype(bf)
    wo = np.asarray(inputs["W_out"], dtype=np.float32).astype(bf)
    b_out = np.asarray(inputs["b_out"], dtype=np.float32)

    xf = x.reshape(2, SL, D)  # batch pairs
    in_maps = []
    for c in range(N_CORES):
        pair = c // 4
        hg = c % 4
        in_maps.append(
            {
                "xT": np.ascontiguousarray(xf[pair].T),
                "wq": np.ascontiguousarray(wq[:, DL * hg : DL * (hg + 1)]),
                "wk": np.ascontiguousarray(wk[:, DL * hg : DL * (hg + 1)]),
                "wv": np.ascontiguousarray(wv[:, DL * hg : DL * (hg + 1)]),
                "wo": np.ascontiguousarray(wo[DL * hg : DL * (hg + 1), :]),
            }
        )

    nc = _get_nc()
    res = run_bass_kernel_spmd(nc, in_maps, core_ids=list(range(N_CORES)), trace=trace)

    y = np.zeros((2, SL, D), dtype=np.float32)
    for c in range(N_CORES):
        y[c // 4] += res.results[c]["y"].astype(np.float32)
    y += b_out[None, None, :]
    out = y.reshape(4, S, D)
    return out, res


def kernel(**inputs) -> np.ndarray:
    out, _ = _run(inputs, trace=False)
    return out


# revision 10
# speedup vs baseline: 1.2753x; 1.0343x over previous
"""Causal multi-head attention on 8 TRN2 NeuronCores.

Problem: x[4, 2048, 2048] @ Wq/Wk/Wv[2048, 2048] -> 16-head causal attention
(head_dim 128) -> out-proj Wo[2048, 2048] + b_out.

Sharding: 4-way head tensor-parallel x 2-way batch data-parallel.
Core c handles head group (c % 4) (4 heads = 512 cols of Wq/Wk/Wv, 512 rows
of Wo) and batch pair (c // 4). Each core emits a partial out-projection for
its 2 batches; the host sums the 4 partials per batch pair (the "all-reduce")
and adds the bias.

Host prep: all inputs pre-cast to bf16 and x pre-transposed to [D, SL] per
batch pair.

Schedule (v2): the kernel is PE-bound overall, but attention alone is paced
by the Scalar engine's exp. So the phases are interleaved per batch to keep
PE saturated:
  A: projections for local batch 0 (chunks 0-3), 16 matmuls per [128,512]
     PSUM unit; chunk 0 consumes wq/xT quarter-DMAs progressively so PE
     starts ~4us in.
  B: batch-0 attention, with batch-1 projection units (48 x 16 matmuls)
     woven between score-pair groups as PE filler (4 per chunk-slot,
     slots 0-11).
  C: batch-1 attention with batch-0 out-projection tiles interleaved
     (p3i), one tile per chunk-slot, its 4 y-matmul groups also woven
     between pairs as filler.
  D: batch-1 out-projection.

Softmax denominator is computed OFF the PE: bf16 pair-sums (DVE) ->
gpsimd add tree -> one [128,512] per chunk -> DVE 32x32 stream-transpose +
grouped free-dim reduce -> [128,16] partials -> tiny DRAM bounce to fold the
4 partition blocks -> DVE reduce/reciprocal -> spread reciprocals staged to
DRAM for the out-projection normalize (PE ones-row broadcast matmul).

ctx matmuls on diagonal sk-tiles are column-restricted (sq >= sk_start);
the skipped columns are exactly the affine_select zeros.
"""

import math

import numpy as np

P = 128
S = 2048          # sequence length
D = 2048          # model dim
NB = 2            # batches per core
SL = NB * S       # local rows (4096)
DL = 512          # local head dims (4 heads x 128)
HL = 4            # local heads
NI = D // P       # 16 i-tiles
SCHUNK = 512
NCHUNK = SL // SCHUNK  # 8
SCALE = 1.0 / math.sqrt(128.0)
N_CORES = 8

_CACHE = {}


def _split_multi_waits(nc):
    """This walrus build accepts at most ONE sync-wait per instruction
    (setupSyncWait: 'Too many sync wait commands'), but Tile emits up to
    ~3 waits per instruction and the kernel-tail drain carries one wait per
    outstanding semaphore. Hoist excess waits onto single-wait nops inserted
    immediately before the instruction on the same engine stream."""
    import bass_rust

    SyncInfo = bass_rust.SyncInfo
    n = 0
    for f in nc.m.functions:
        for b in f.blocks:
            out = []
            changed = False
            for inst in list(b.instructions):
                si = getattr(inst, "sync_info", None)
                if si is not None and si.on_wait and len(si.on_wait) > 1:
                    waits = list(si.on_wait)
                    for w in waits[:-1]:
                        n += 1
                        nop = bass_rust.InstNoOp(
                            name=f"waitsplit-{n}", ins=[], outs=[]
                        )
                        nop.engine = inst.engine
                        nop.sync_info = SyncInfo(on_wait=[w], on_update=[])
                        out.append(nop)
                    inst.sync_info = SyncInfo(
                        on_wait=[waits[-1]], on_update=list(si.on_update or [])
                    )
                    changed = True
                out.append(inst)
            if changed:
                b.instructions = out


def _build():
    import concourse.bass as bass
    import concourse.mybir as mybir
    import concourse.tile as tile
    from concourse.masks import make_identity

    f32 = mybir.dt.float32
    bf16 = mybir.dt.bfloat16

    nc = bass.Bass()
    x_in = nc.declare_dram_parameter("xT", [D, SL], bf16, isOutput=False)
    wq_in = nc.declare_dram_parameter("wq", [D, DL], bf16, isOutput=False)
    wk_in = nc.declare_dram_parameter("wk", [D, DL], bf16, isOutput=False)
    wv_in = nc.declare_dram_parameter("wv", [D, DL], bf16, isOutput=False)
    wo_in = nc.declare_dram_parameter("wo", [DL, D], bf16, isOutput=False)
    y_out = nc.declare_dram_parameter("y", [SL, D], bf16, isOutput=True)

    with tile.TileContext(nc) as tc:
        _emit(nc, tc, mybir, make_identity, x_in, wq_in, wk_in, wv_in, wo_in, y_out)
    _split_multi_waits(nc)
    return nc


def _emit(nc, tc, mybir, make_identity, x_in, wq_in, wk_in, wv_in, wo_in, y_out):
    from contextlib import ExitStack

    f32 = mybir.dt.float32
    bf16 = mybir.dt.bfloat16
    Exp = mybir.ActivationFunctionType.Exp
    X = mybir.AxisListType.X
    ADD = mybir.AluOpType.add

    ctx = ExitStack()
    with ctx:
        dram = ctx.enter_context(tc.tile_pool(name="dram", bufs=1, space="DRAM"))
        consts = ctx.enter_context(tc.tile_pool(name="consts", bufs=1))
        wpool = ctx.enter_context(tc.tile_pool(name="wpool", bufs=1))
        xt_pool = ctx.enter_context(tc.tile_pool(name="xt_pool", bufs=3))
        qkv_pool = ctx.enter_context(tc.tile_pool(name="qkv_pool", bufs=3))
        att_pool = ctx.enter_context(tc.tile_pool(name="att_pool", bufs=2))
        out_pool = ctx.enter_context(tc.tile_pool(name="out_pool", bufs=3))
        # PSUM budget (8 banks of [128, 2KB]):
        #   pbig  2 x [128,1024] f32 = 4 banks  (scores; D-phase y pairs)
        #   psmall 2 x [128,512] f32 = 2 banks  (pctx; C bcp; D pya/pyb)
        #   pone  2 x [128,512] f32 = 2 banks  (P1 units; C p3i y; D bcp)
        pbig = ctx.enter_context(tc.tile_pool(name="pbig", bufs=2, space="PSUM"))
        psmall = ctx.enter_context(tc.tile_pool(name="psmall", bufs=2, space="PSUM"))
        pone = ctx.enter_context(tc.tile_pool(name="pone", bufs=2, space="PSUM"))

        # DRAM staging for q/k/v (transposed layouts), ctx, den partials
        qT_d = dram.tile([DL, SL], bf16, name="qT_d")
        kT_d = dram.tile([DL, SL], bf16, name="kT_d")
        v_d = dram.tile([P, SL // P, DL], bf16, name="v_d")
        cT_d = dram.tile([DL, SL], bf16, name="cT_d")
        recb_d = dram.tile([NB, HL, S], bf16, name="recb_d")

        qT_r = qT_d.rearrange("(a p) s -> p a s", p=P)   # [128, 4, 4096]
        kT_r = kT_d.rearrange("(a p) s -> p a s", p=P)
        v_r = v_d                                        # [128, 32, 512]
        cT_r = cT_d.rearrange("(a p) s -> p a s", p=P)

        ones1 = consts.tile([1, P], bf16, name="ones1")
        nc.vector.memset(ones1, 1.0)
        # fold mask M[p, u] = 1 iff p %% 32 == u (for the den partition-block
        # fold matmul): sum the 32-col groups of a 128x128 identity.
        ident, ident_free = tc.tile([P, P], bf16, name="ident")
        make_identity(nc, ident)
        mfold_f, mfold_f_free = tc.tile([P, 32], f32, name="mfold_f")
        nc.vector.tensor_reduce(
            mfold_f, ident.rearrange("p (a u) -> p u a", a=4), axis=X, op=ADD
        )
        mfold = consts.tile([P, 32], bf16, name="mfold")
        nc.vector.tensor_copy(mfold, mfold_f)
        mfold_f_free()
        ident_free()

        # --- weights: bf16 DMA into SBUF, 4 i-quarters each (scalar queue) ---
        wq_sb = wpool.tile([P, NI, DL], bf16, name="wq_sb")
        wk_sb = wpool.tile([P, NI, DL], bf16, name="wk_sb")
        wv_sb = wpool.tile([P, NI, DL], bf16, name="wv_sb")
        wo_sb = wpool.tile([P, HL, D], bf16, name="wo_sb")

        def emit_w(w_in, w_sb):
            w_r = w_in.rearrange("(a p) d -> p a d", p=P)  # [128, 16, 512]
            for g in range(4):
                nc.scalar.dma_start(
                    out=w_sb[:, 4 * g : 4 * g + 4, :],
                    in_=w_r[:, 4 * g : 4 * g + 4, :],
                )

        def emit_wo():
            for dt in range(HL):
                nc.scalar.dma_start(
                    out=wo_sb[:, dt, :],
                    in_=wo_in[P * dt : P * (dt + 1), :],
                )

        # --- P1 projection units (16 matmuls into one [128,512] PSUM) ---
        xT_r = x_in.rearrange("(a p) s -> p a s", p=P)  # [128, 16, 4096]
        xt_tiles = {}

        def load_xt(ch, parts=1):
            xT = xt_pool.tile([P, NI, SCHUNK], bf16, name="xT", tag="xT")
            step = NI // parts
            for g in range(parts):
                nc.sync.dma_start(
                    out=xT[:, step * g : step * (g + 1), :],
                    in_=xT_r[
                        :, step * g : step * (g + 1),
                        SCHUNK * ch : SCHUNK * (ch + 1),
                    ],
                )
            xt_tiles[ch] = xT

        def p1_qk_unit(ch, w_sb, out_r, ht):
            xT = xt_tiles[ch]
            pq = pone.tile([P, SCHUNK], f32, name="pq1", tag="po")
            for i in range(NI):
                nc.tensor.matmul(
                    pq,
                    lhsT=w_sb[:, i, P * ht : P * (ht + 1)],
                    rhs=xT[:, i, :],
                    start=(i == 0),
                    stop=(i == NI - 1),
                )
            qsb = qkv_pool.tile([P, SCHUNK], bf16, name="qsb", tag="qsb", bufs=3)
            nc.scalar.copy(qsb, pq)
            nc.scalar.dma_start(
                out=out_r[:, ht, SCHUNK * ch : SCHUNK * (ch + 1)], in_=qsb
            )

        def p1_v_unit(ch, st):
            xT = xt_tiles[ch]
            pv = pone.tile([P, DL], f32, name="pv1", tag="po")
            for i in range(NI):
                nc.tensor.matmul(
                    pv,
                    lhsT=xT[:, i, P * st : P * (st + 1)],
                    rhs=wv_sb[:, i, :],
                    start=(i == 0),
                    stop=(i == NI - 1),
                )
            vsb = qkv_pool.tile([P, DL], bf16, name="vsb", tag="qsb", bufs=3)
            nc.scalar.copy(vsb, pv)
            nc.scalar.dma_start(out=v_r[:, 4 * ch + st, :], in_=vsb)

        # --- Phase A: batch-0 projections (chunks 0-3) ---
        load_xt(0, parts=4)
        emit_w(wq_in, wq_sb)
        for ch in range(4):
            for ht in range(HL):
                p1_qk_unit(ch, wq_sb, qT_r, ht)
                if ch == 0 and ht == 1:
                    emit_w(wk_in, wk_sb)
            if ch + 1 < 4:
                load_xt(ch + 1)
            for ht in range(HL):
                p1_qk_unit(ch, wk_sb, kT_r, ht)
                if ch == 0 and ht == 1:
                    emit_w(wv_in, wv_sb)
                if ch == 1 and ht == 1:
                    emit_wo()
            for st in range(HL):
                p1_v_unit(ch, st)
        load_xt(4)

        # --- batch-1 projection units, drained as PE filler in phase B ---
        unit_queue = []
        for ch in range(4, NCHUNK):
            for ht in range(HL):
                unit_queue.append(("q", ch, ht))
            for ht in range(HL):
                unit_queue.append(("k", ch, ht))
            for st in range(HL):
                unit_queue.append(("v", ch, st))
        uq_pos = [0]

        def emit_unit():
            if uq_pos[0] >= len(unit_queue):
                return False
            kind, ch, j = unit_queue[uq_pos[0]]
            uq_pos[0] += 1
            if kind == "q":
                p1_qk_unit(ch, wq_sb, qT_r, j)
            elif kind == "k":
                p1_qk_unit(ch, wk_sb, kT_r, j)
            else:
                p1_v_unit(ch, j)
            return True

        # --- attention prefetch machinery ---
        bh_list = [(b, h) for b in range(NB) for h in range(HL)]
        ktb_tiles = {}
        vtb_tiles = {}

        def load_ktb(i):
            b, h = bh_list[i]
            ktb = att_pool.tile([P, S], bf16, name="ktb", tag="ktb")
            nc.sync.dma_start(
                out=ktb, in_=kT_d[P * h : P * (h + 1), S * b : S * (b + 1)]
            )
            ktb_tiles[i] = ktb

        def load_vtb(b):
            vtb_all = att_pool.tile([P, S // P, DL], bf16, name="vtb", tag="vtb")
            nc.sync.dma_start(
                out=vtb_all,
                in_=v_r[:, (S // P) * b : (S // P) * (b + 1), :],
            )
            vtb_tiles[b] = vtb_all

        qtc_tiles = {}
        cq_list = [
            (bh_i, c) for bh_i in range(len(bh_list)) for c in range(S // SCHUNK)
        ]

        def load_qtc(i):
            bh_i, c = cq_list[i]
            b, h = bh_list[bh_i]
            qtc = att_pool.tile([P, SCHUNK], bf16, name="qtc", tag="qtc", bufs=3)
            nc.sync.dma_start(
                out=qtc,
                in_=qT_d[
                    P * h : P * (h + 1),
                    S * b + SCHUNK * c : S * b + SCHUNK * (c + 1),
                ],
            )
            qtc_tiles[i] = qtc

        load_ktb(0)
        load_vtb(0)
        load_qtc(0)
        load_qtc(1)
        pend = [None]
        pend_pairs = []
        pend_late = []  # [countdown, fn]; fires when countdown reaches 0

        # --- P3 interleave (batch-0 out-projection during batch-1 attn) ---
        p3i_ctn = {}

        def p3i_load(t):
            ctb = out_pool.tile([P, HL, P], bf16, name="ctb", tag="ctb", bufs=4)
            nc.sync.dma_start(out=ctb, in_=cT_r[:, :, P * t : P * (t + 1)])
            rrow = out_pool.tile([1, HL * P], bf16, name="rrow", tag="rrow", bufs=4)
            nc.sync.dma_start(
                out=rrow.rearrange("q (a s) -> q a s", a=HL),
                in_=recb_d[0, :, P * t : P * (t + 1)].rearrange("a s -> () a s"),
            )
            p3i_ctn[t] = (ctb, rrow)

        def p3i_top(t):
            ctb, rrow = p3i_ctn[t]
            bcp = psmall.tile([P, 512], f32, name="p3bcp", tag="ps")
            nc.tensor.matmul(
                bcp[:, : HL * P], lhsT=ones1, rhs=rrow, start=True, stop=True
            )
            ctn = out_pool.tile([P, HL, P], bf16, name="ctn", tag="ctn", bufs=4)
            nc.vector.tensor_mul(ctn, ctb, bcp.rearrange("p (a s) -> p a s", a=HL))
            p3i_ctn[t] = ctn

        def p3i_f_thunk(t, f):
            def th():
                ctn = p3i_ctn[t]
                py = pone.tile([P, 512], f32, name="p3y", tag="po")
                for dt in range(HL):
                    nc.tensor.matmul(
                        py,
                        lhsT=ctn[:, dt, :],
                        rhs=wo_sb[:, dt, 512 * f : 512 * (f + 1)],
                        start=(dt == 0),
                        stop=(dt == HL - 1),
                    )
                ysb2 = out_pool.tile([P, 512], bf16, name="ysb2", tag="ysb2", bufs=4)
                if f == 0:
                    nc.scalar.copy(ysb2, py)
                    nc.scalar.dma_start(
                        out=y_out[P * t : P * (t + 1), 512 * f : 512 * (f + 1)],
                        in_=ysb2,
                    )
                else:
                    nc.vector.tensor_copy(ysb2, py)

                    def y_store(ysb2=ysb2, t=t, f=f):
                        nc.sync.dma_start(
                            out=y_out[
                                P * t : P * (t + 1), 512 * f : 512 * (f + 1)
                            ],
                            in_=ysb2,
                        )
                    pend_late.append([2, y_store])
            return th

        def flush_pend():
            # deferred work (reciprocal path, sync-queue stores) fires a fixed
            # number of flushes after being queued, so by the time a store
            # reaches the head of the sync DGE its producer has executed and
            # it never convoys the prefetch loads behind it.
            ready = [e for e in pend_late if e[0] <= 0]
            pend_late[:] = [e for e in pend_late if e[0] > 0]
            for e in pend_late:
                e[0] -= 1
            if pend[0] is not None:
                pend[0]()
                pend[0] = None
            for e in ready:
                e[1]()

        # --- main attention loop (phases B and C) ---
        for bh_i, (b, h) in enumerate(bh_list):
            ktb = ktb_tiles.pop(bh_i)
            vtb_all = vtb_tiles[b]
            for c in range(S // SCHUNK):  # 4 sq-chunks
                slot = 4 * bh_i + c
                if slot in (0, 4, 8):
                    load_xt(5 + slot // 4)
                if c == 2 and bh_i + 1 < len(bh_list):
                    load_ktb(bh_i + 1)
                if bh_i == 3 and c == 0 and b + 1 < NB:
                    load_vtb(b + 1)
                cq_i = 4 * bh_i + c
                if cq_i + 2 < len(cq_list):
                    load_qtc(cq_i + 2)
                qtc = qtc_tiles.pop(cq_i)
                p3t = 4 * (bh_i - HL) + c if bh_i >= HL else None
                fillers = []
                if p3t is not None:
                    p3i_top(p3t)
                    fillers = [p3i_f_thunk(p3t, f) for f in range(4)]
                elif slot < 12:
                    fillers = [emit_unit] * 4
                pctx = psmall.tile([P, SCHUNK], f32, name="pctx", tag="ps")
                jmax = 4 * c + 4  # sk-tiles with sk_start <= sq_end
                npairs = jmax // 2
                j0s = [2 * k for k in range(npairs)]
                chunk_ds = []

                def emit_av_group(items, pctx=pctx, vtb_all=vtb_all, h=h,
                                  c=c, npairs=npairs, chunk_ds=chunk_ds):
                    # ctx matmuls back-to-back (same PSUM target), columns
                    # restricted on diagonal tiles to sq >= sk_start (the
                    # skipped columns hold affine_select zeros).
                    for at2, dsum, j0, pi in items:
                        for j2 in range(2):
                            j = j0 + j2
                            off = max(0, P * (j - 4 * c))
                            nc.tensor.matmul(
                                pctx[:, off:],
                                lhsT=vtb_all[:, j, P * h : P * (h + 1)],
                                rhs=at2[:, 512 * j2 + off : 512 * (j2 + 1)],
                                start=(pi == 0 and j2 == 0),
                                stop=(pi == npairs - 1 and j2 == 1),
                            )
                    # level-2 pair-sum on gpsimd (den tree stays off DVE/PE)
                    (_, dsa, _, _), (_, dsb, _, _) = items
                    dsum2 = att_pool.tile(
                        [P, 512], bf16, name="dsum2", tag="dsum2", bufs=4
                    )
                    nc.gpsimd.tensor_add(dsum2, dsa, dsb)
                    chunk_ds.append(dsum2)

                def emit_tail(pctx=pctx, b=b, h=h, c=c, chunk_ds=chunk_ds):
                    csb = att_pool.tile([P, SCHUNK], bf16, name="csb",
                                        tag="csb", bufs=3)
                    nc.scalar.copy(csb, pctx)
                    nc.scalar.dma_start(
                        out=cT_d[
                            P * h : P * (h + 1),
                            S * b + SCHUNK * c : S * b + SCHUNK * (c + 1),
                        ],
                        in_=csb,
                    )
                    # gpsimd tree -> one [128,512] of sk-partials
                    lvl = list(chunk_ds)
                    while len(lvl) > 1:
                        nxt = []
                        for k in range(0, len(lvl) - 1, 2):
                            t = att_pool.tile([P, 512], bf16, name="dtree",
                                              tag="dtree", bufs=3)
                            nc.gpsimd.tensor_add(t, lvl[k], lvl[k + 1])
                            nxt.append(t)
                        if len(lvl) % 2:
                            nxt.append(lvl[-1])
                        lvl = nxt
                    dtot = lvl[0]
                    # DVE: 32x32 block transpose + grouped free reduce
                    # -> z[32a+u, g] = sum over partition block a of
                    #    column q = 32g+u
                    yst = att_pool.tile([P, 512], bf16, name="yst",
                                        tag="yst", bufs=2)
                    nc.vector.transpose(yst, dtot)
                    z = att_pool.tile([P, 16], f32, name="zred",
                                      tag="zred", bufs=2)
                    nc.vector.tensor_reduce(
                        z, yst.rearrange("p (g u) -> p g u", u=32),
                        axis=X, op=ADD,
                    )
                    zb = att_pool.tile([P, 16], bf16, name="zb",
                                       tag="zb", bufs=2)
                    nc.gpsimd.tensor_copy(zb, z)
                    # fold the 4 partition blocks on the PE: a 16-column
                    # mask matmul (den_ps[u, g] = sum_a z[32a+u, g])
                    den_ps = pone.tile([32, 16], f32, name="den_ps", tag="po")
                    nc.tensor.matmul(
                        den_ps, lhsT=mfold, rhs=zb, start=True, stop=True
                    )

                    def tail_late(den_ps=den_ps, b=b, h=h, c=c):
                        rsp = att_pool.tile([32, 16], f32, name="rsp",
                                            tag="rsp", bufs=2)
                        nc.vector.reciprocal(rsp, den_ps)
                        rspb = att_pool.tile([32, 16], bf16, name="rspb",
                                             tag="rspb", bufs=2)
                        nc.vector.tensor_copy(rspb, rsp)

                        def rec_store(rspb=rspb, b=b, h=h, c=c):
                            nc.sync.dma_start(
                                out=recb_d[
                                    b, h, SCHUNK * c : SCHUNK * (c + 1)
                                ].rearrange("(g p) -> p g", p=32),
                                in_=rspb,
                            )
                        pend_late.append([2, rec_store])
                    pend_late.append([0, tail_late])

                for pi, j0 in enumerate(j0s):
                    ps2 = pbig.tile([P, 1024], f32, name="ps2", tag="pb")
                    for j2 in range(2):
                        j = j0 + j2
                        off = max(0, P * (j - 4 * c))
                        nc.tensor.matmul(
                            ps2[:, 512 * j2 + off : 512 * (j2 + 1)],
                            lhsT=ktb[:, P * j : P * (j + 1)],
                            rhs=qtc[:, off:],
                            start=True,
                            stop=True,
                        )
                    at2 = att_pool.tile([P, 1024], bf16, name="at2",
                                        tag="at2", bufs=5)
                    nc.scalar.activation(at2, ps2, Exp, scale=SCALE)
                    if j0 >= 4 * c:  # diagonal pair: zero sk > sq
                        nc.gpsimd.affine_select(
                            out=at2.rearrange("p (a b) -> p a b", a=2),
                            in_=at2.rearrange("p (a b) -> p a b", a=2),
                            compare_op=mybir.AluOpType.is_ge,
                            fill=0.0,
                            base=(0 if j0 == 4 * c else -256),
                            channel_multiplier=-1,
                            pattern=[[-P, 2], [1, 512]],
                        )
                    dsum = att_pool.tile([P, 512], bf16, name="dsum",
                                         tag="dsum", bufs=4)
                    nc.vector.tensor_add(dsum, at2[:, :512], at2[:, 512:])
                    # PE filler between pair groups: batch-1 projection
                    # units (B) or p3i y-matmul groups (C) cover the exp
                    # latency of the deferred ctx group.
                    if fillers:
                        fillers.pop(0)()
                    flush_pend()
                    pend_pairs.append((at2, dsum, j0, pi))
                    if len(pend_pairs) == 2:
                        items = list(pend_pairs)
                        pend_pairs.clear()
                        is_last = pi == npairs - 1

                        def pend_fn(items=items, emit=emit_av_group,
                                    tail=(emit_tail if is_last else None)):
                            emit(items)
                            if tail is not None:
                                tail()

                        pend[0] = pend_fn
                for th in fillers:
                    th()
                nxt = 0 if (bh_i == HL - 1 and c == 3) else (
                    p3t + 1 if p3t is not None else None
                )
                if nxt is not None and nxt < S // P:
                    p3i_load(nxt)
        for _ in range(5):
            flush_pend()

        # --- Phase D: batch-1 out-projection ---
        bt_list = [(1, t) for t in range(S // P)]

        def p3_load(i):
            b, t = bt_list[i]
            col0 = S * b + P * t
            ctb = out_pool.tile([P, HL, P], bf16, name="ctb", tag="ctb", bufs=4)
            nc.sync.dma_start(out=ctb, in_=cT_r[:, :, col0 : col0 + P])
            rrow = out_pool.tile([1, HL * P], bf16, name="rrow", tag="rrow", bufs=4)
            nc.sync.dma_start(
                out=rrow.rearrange("q (a s) -> q a s", a=HL),
                in_=recb_d[b, :, P * t : P * (t + 1)].rearrange("a s -> () a s"),
            )
            bcp = pone.tile([P, 512], f32, name="bcp", tag="po")
            nc.tensor.matmul(
                bcp[:, : HL * P], lhsT=ones1, rhs=rrow, start=True, stop=True
            )
            ctn = out_pool.tile([P, HL, P], bf16, name="ctn", tag="ctn", bufs=4)
            nc.vector.tensor_mul(
                ctn, ctb, bcp.rearrange("p (a s) -> p a s", a=HL)
            )
            return ctn

        ctn_next = p3_load(0)
        for i, (b, t) in enumerate(bt_list):
            ctn = ctn_next
            if i + 1 < len(bt_list):
                ctn_next = p3_load(i + 1)
            col0 = S * b + P * t
            use_small = (i % 2 == 0)
            for fp in range(2):  # pairs of f-chunks
                if use_small:
                    pya = psmall.tile([P, 512], f32, name="pya", tag="ps")
                    pyb = psmall.tile([P, 512], f32, name="pyb", tag="ps")
                    halves = (pya, pyb)
                else:
                    py = pbig.tile([P, 1024], f32, name="py", tag="pb")
                    halves = (py[:, :512], py[:, 512:])
                for f2 in range(2):
                    f = 2 * fp + f2
                    for dt in range(HL):
                        nc.tensor.matmul(
                            halves[f2],
                            lhsT=ctn[:, dt, :],
                            rhs=wo_sb[:, dt, 512 * f : 512 * (f + 1)],
                            start=(dt == 0),
                            stop=(dt == HL - 1),
                        )
                ysb = out_pool.tile([P, 1024], bf16, name="ysb", tag="ysb", bufs=3)
                if fp == 0:
                    nc.vector.tensor_copy(ysb[:, :512], halves[0])
                    nc.vector.tensor_copy(ysb[:, 512:], halves[1])
                else:
                    nc.scalar.copy(ysb[:, :512], halves[0])
                    nc.scalar.copy(ysb[:, 512:], halves[1])
                nc.scalar.dma_start(
                    out=y_out[col0 : col0 + P, 1024 * fp : 1024 * (fp + 1)],
                    in_=ysb,
                )


def _get_nc():
    if "nc" not in _CACHE:
        _CACHE["nc"] = _build()
    return _CACHE["nc"]


def _run(inputs, trace=False):
    import ml_dtypes

    from concourse.bass_utils import run_bass_kernel_spmd

    bf = ml_dtypes.bfloat16
    x = np.asarray(inputs["x"], dtype=np.float32).astype(bf)
    wq = np.asarray(inputs["W_query"], dtype=np.float32).astype(bf)
    wk = np.asarray(inputs["W_key"], dtype=np.float32).astype(bf)
    wv = np.asarray(inputs["W_value"], dtype=np.float32).astype(bf)
    wo = np.asarray(inputs["W_out"], dtype=np.float32).astype(bf)
    b_out = np.asarray(inputs["b_out"], dtype=np.float32)

    xf = x.reshape(2, SL, D)  # batch pairs
    in_maps = []
    for c in range(N_CORES):
        pair = c // 4
        hg = c % 4
        in_maps.append(
            {
                "xT": np.ascontiguousarray(xf[pair].T),
                "wq": np.ascontiguousarray(wq[:, DL * hg : DL * (hg + 1)]),
                "wk": np.ascontiguousarray(wk[:, DL * hg : DL * (hg + 1)]),
                "wv": np.ascontiguousarray(wv[:, DL * hg : DL * (hg + 1)]),
                "wo": np.ascontiguousarray(wo[DL * hg : DL * (hg + 1), :]),
            }
        )

    nc = _get_nc()
    res = run_bass_kernel_spmd(nc, in_maps, core_ids=list(range(N_CORES)), trace=trace)

    y = np.zeros((2, SL, D), dtype=np.float32)
    for c in range(N_CORES):
        y[c // 4] += res.results[c]["y"].astype(np.float32)
    y += b_out[None, None, :]
    out = y.reshape(4, S, D)
    return out, res


def kernel(**inputs) -> np.ndarray:
    out, _ = _run(inputs, trace=False)
    return out
